# revision 1
# baseline (speedup 1.0000x reference)
"""Trainium2 Bass kernel for per-image masked-softmax entropy (EntropyLoss).

Math (per (n, c) segment, over the HW=512*512 elements x of heatmap[n, c]):
    mask  = x > 0
    softmax over the masked elements, entropy in bits, summed over c and
    divided by the total positive count of image n.

The entropy of a masked softmax is invariant to the stabilizing shift m, so
we may use m = 0 (randn inputs keep exp(x) <= ~e^6, no overflow):
    S_c   = sum_{x>0} exp(x)
    U_c   = sum_{x>0} x * exp(x)
    ent_c = (log S_c - U_c / S_c) / ln2          [bits]
    out_n = sum_c ent_c / sum_c count_c

Device work per segment tile [128, 2048] (bf16 x, cast during DMA):
    r  = relu(x)                 (DVE tensor_scalar, 4x bf16)
    a  = exp(r)                  (ACT, fused accum -> S'_c = S_c + #nonpos)
    w  = a * r                   (DVE tensor_tensor, 2x bf16)
    mk = x > 0                   (DVE tensor_scalar, 4x bf16)
    U_c, count_c                 (PE: one-hot stationary weights route each
                                  segment's column sums into PSUM row c of a
                                  single [20, 512] accumulator; one final
                                  tensor_reduce folds 512 -> 1 for all rows)
S_c is recovered on the host as S'_c - (HW - count_c) since exp(0) = 1 for
every non-positive element. Final log/divide runs on host in float64 over
60 scalars per core.
"""

import os

import numpy as np

N, C, H, W = 8, 20, 512, 512
HW = H * W
P = 128
F = HW // P  # 2048
NCORES = 8
LN2 = 0.6931471805599453

DATA_BUFS = int(os.environ.get("ENTROPY_DATA_BUFS", "8"))
# Segments loaded as fp32 via HWDGE queues and cast to bf16 on ACT; the
# rest stream through SWDGE cast-DMAs. Measured: the cast-DMA stream alone
# already saturates HBM (~342 GB/s read), so the hybrid only hurts — keep 0.
NHW = int(os.environ.get("ENTROPY_NHW", "0"))
WARM_MM = int(os.environ.get("ENTROPY_WARM_MM", "24"))

_CACHE = {}


def _build_program():
    import concourse.bacc as bacc
    import concourse.mybir as mybir
    import concourse.tile as tile

    dt = mybir.dt
    Alu = mybir.AluOpType
    Act = mybir.ActivationFunctionType

    nc = bacc.Bacc(None, target_bir_lowering=False, debug=False)

    x_dram = nc.dram_tensor("x", [C, P, F], dt.float32, kind="ExternalInput")
    s_dram = nc.dram_tensor("s_out", [P, C + 2], dt.float32, kind="ExternalOutput")
    u_dram = nc.dram_tensor("u_out", [C, 1], dt.float32, kind="ExternalOutput")
    n_dram = nc.dram_tensor("n_out", [C, 1], dt.float32, kind="ExternalOutput")

    with tile.TileContext(nc) as tc:
        with (
            tc.tile_pool(name="const", bufs=1) as constp,
            tc.tile_pool(name="res", bufs=1) as resp,
            tc.tile_pool(name="data", bufs=DATA_BUFS) as datap,
            tc.tile_pool(name="psum", bufs=1, space="PSUM") as psump,
        ):
            # Sliding-window one-hot weights: oh[:, 20 - c : 40 - c] is a
            # [128, 20] matrix whose only nonzero column (all ones) is c.
            oh = constp.tile([P, 2 * C], dt.bfloat16)
            nc.gpsimd.memset(oh[:], 0.0)
            nc.gpsimd.memset(oh[:, C : C + 1], 1.0)

            # Two extra accum columns: segments 0 and 1 are processed as
            # half-width items (earlier pipeline start); their second
            # halves accumulate into columns C and C+1 (host adds them).
            s_res = resp.tile([P, C + 2], dt.float32)
            u_red = resp.tile([C, 1], dt.float32)
            n_red = resp.tile([C, 1], dt.float32)

            u_psum = psump.tile([C, 512], dt.float32)
            c_psum = psump.tile([C, 512], dt.float32)

            # PE warmup: dummy matmuls during the DMA fill phase keep the
            # PE busy so HAM upclocks it to 2.4 GHz before real work lands.
            if WARM_MM:
                warm = constp.tile([P, 512], dt.bfloat16)
                nc.gpsimd.memset(warm[:], 0.0)
                w_psum = psump.tile([C, 512], dt.float32)
                for i in range(WARM_MM):
                    nc.tensor.matmul(
                        w_psum[:], oh[:, 0:C], warm[:],
                        start=(i == 0), stop=(i == WARM_MM - 1),
                    )

            # Work items: (segment, dram col offset, width, S' accum col).
            # First two segments split in half for an earlier pipeline start.
            items = [
                (0, 0, F // 2, 0), (0, F // 2, F // 2, C),
                (1, 0, F // 2, 1), (1, F // 2, F // 2, C + 1),
            ] + [(c, 0, F, c) for c in range(2, C)]

            for idx, (c, lo, width, scol) in enumerate(items):
                x_t = datap.tile([P, width], dt.bfloat16, tag="x")
                # SWDGE DMA casts fp32 -> bf16 on the fly.
                nc.gpsimd.dma_start(x_t[:], x_dram[c, :, lo : lo + width])

                r_t = datap.tile([P, width], dt.bfloat16, tag="r")
                a_t = datap.tile([P, width], dt.bfloat16, tag="a")
                w_t = datap.tile([P, width], dt.bfloat16, tag="w")
                mk_t = datap.tile([P, width], dt.bfloat16, tag="mk")

                nc.vector.tensor_scalar(r_t[:], x_t[:], 0.0, None, Alu.max)
                nc.scalar.activation(
                    a_t[:], r_t[:], Act.Exp, accum_out=s_res[:, scol : scol + 1]
                )
                nc.vector.tensor_tensor(w_t[:], a_t[:], r_t[:], Alu.mult)
                nc.vector.tensor_scalar(mk_t[:], x_t[:], 0.0, None, Alu.is_gt)

                lhsT = oh[:, C - c : 2 * C - c]
                first = idx == 0
                last = idx == len(items) - 1
                nj = width // 512
                for j in range(nj):
                    nc.tensor.matmul(
                        u_psum[:],
                        lhsT,
                        w_t[:, j * 512 : (j + 1) * 512],
                        start=(first and j == 0),
                        stop=(last and j == nj - 1),
                    )
                for j in range(nj):
                    nc.tensor.matmul(
                        c_psum[:],
                        lhsT,
                        mk_t[:, j * 512 : (j + 1) * 512],
                        start=(first and j == 0),
                        stop=(last and j == nj - 1),
                    )

            nc.vector.tensor_reduce(
                u_red[:], u_psum[:], mybir.AxisListType.X, Alu.add
            )
            nc.vector.tensor_reduce(
                n_red[:], c_psum[:], mybir.AxisListType.X, Alu.add
            )
            nc.sync.dma_start(s_dram[:], s_res[:])
            nc.sync.dma_start(u_dram[:], u_red[:])
            nc.sync.dma_start(n_dram[:], n_red[:])

    nc.compile()
    return nc


def _get_program():
    if "nc" not in _CACHE:
        _CACHE["nc"] = _build_program()
    return _CACHE["nc"]


def _run(heatmap: np.ndarray, trace: bool = False):
    from concourse.bass_utils import run_bass_kernel_spmd

    nc = _get_program()
    in_maps = [
        {"x": np.ascontiguousarray(heatmap[i].reshape(C, P, F), dtype=np.float32)}
        for i in range(NCORES)
    ]
    return run_bass_kernel_spmd(nc, in_maps, list(range(NCORES)), trace=trace)


def _finalize(results) -> np.ndarray:
    """Host epilogue: a few scalars per core -> entropy[n] in float64."""
    out = np.zeros(N, dtype=np.float64)
    for n in range(NCORES):
        r = results[n]
        s_full = r["s_out"].astype(np.float64).sum(axis=0)   # [C + 2]
        s_prime = s_full[:C]
        s_prime[0] += s_full[C]
        s_prime[1] += s_full[C + 1]
        cnt = r["n_out"].astype(np.float64).reshape(C)       # [C]
        u = r["u_out"].astype(np.float64).reshape(C)         # [C]
        s = s_prime - (HW - cnt)                             # masked sum exp
        ent = np.zeros(C, dtype=np.float64)
        ok = s > 0
        ent[ok] = (np.log(s[ok]) - u[ok] / s[ok]) / LN2
        out[n] = ent.sum() / cnt.sum()
    return out.astype(np.float32)


def kernel(heatmap: np.ndarray) -> np.ndarray:
    heatmap = np.asarray(heatmap, dtype=np.float32)
    assert heatmap.shape == (N, C, H, W), heatmap.shape
    res = _run(heatmap, trace=False)
    return _finalize(res.results)

